# revision 7
# baseline (speedup 1.0000x reference)
"""FFT causal long-conv (H3/Hyena fftconv) as a blocked-Toeplitz matmul kernel
for 8 Trainium2 NeuronCores.

Math: y[b,d,l] = sum_{t<=l} filter[d,t] * x[b,d,l-t]  (causal conv, L taps).

Instead of an on-device FFT, the causal conv is computed directly as a
lower-block-triangular Toeplitz matmul: with 128-wide blocks (J=L/128 blocks),
y_i = sum_{k<=i} T_k @ x_{i-k} where T_k[a,c] = f[128k + a - c].  The T_k are
materialized host-side (bf16) as PE-ready lhsT tiles, so the device does only
dense [128,128]x[128,N] matmuls accumulating in fp32 PSUM — no transposes,
no twiddles.  MAC count is L^2/2 per (b,d) pair, which at L=4096 beats a
two-stage FFT factorization, and every DMA is a clean contiguous transfer.

Sharding: channels D=1024 split 128 per core (data-parallel over D, per the
independence of each channel's conv); all B=16 batches stay on-core so each
matmul gets the full N=512 free dim.
"""

import numpy as np
import ml_dtypes

B, D, L = 16, 1024, 4096
NCORES = 8
DC = D // NCORES  # channels per core
C = 128           # time-block size == PE contraction dim
J = L // C        # 32 time blocks
N = J * B         # 512 = matmul free dim (j-block outer, batch inner)
GROUP = 8         # channels per DMA batch

BF16 = ml_dtypes.bfloat16

_CACHE = {}


def _build_nc():
    if "nc" in _CACHE:
        return _CACHE["nc"]

    import concourse.bacc as bacc
    import concourse.tile as tile
    import concourse.mybir as mybir

    nc = bacc.Bacc("TRN2", target_bir_lowering=False, debug=False, num_devices=NCORES)

    # Layouts are chosen so every DMA has long contiguous per-partition runs:
    #   xt[c, d, n]    n = j*B + b         (input, time-within-block on partitions)
    #   ft[c, d, k, a] = f[d, 128k + a - c] (PE-ready lhsT Toeplitz tiles)
    #   yt[a, d, n]    n = i*B + b         (output)
    xt = nc.dram_tensor("xt", [C, DC, N], mybir.dt.bfloat16, kind="ExternalInput")
    ft = nc.dram_tensor("ft", [C, DC, J, C], mybir.dt.bfloat16, kind="ExternalInput")
    yt = nc.dram_tensor("yt", [C, DC, N], mybir.dt.bfloat16, kind="ExternalOutput")

    with tile.TileContext(nc) as tc:
        with (
            tc.tile_pool(name="wpool", bufs=2) as wpool,
            tc.tile_pool(name="xpool", bufs=3) as xpool,
            tc.tile_pool(name="ypool", bufs=3) as ypool,
            tc.tile_pool(name="pspool", bufs=8, space="PSUM") as pspool,
        ):
            for g in range(DC // GROUP):
                sl = slice(g * GROUP, (g + 1) * GROUP)
                # Alternate the two HWDGE rings (SP / ACT) between groups so
                # consecutive groups' transfers overlap; keep everything off
                # the slow gpsimd SWDGE path.
                eng_a = nc.sync if g % 2 == 0 else nc.scalar
                eng_b = nc.scalar if g % 2 == 0 else nc.sync
                xg = xpool.tile([C, GROUP, N], mybir.dt.bfloat16)
                eng_b.dma_start(out=xg, in_=xt[:, sl, :])
                wt = wpool.tile([C, GROUP, J, C], mybir.dt.bfloat16)
                eng_a.dma_start(out=wt, in_=ft[:, sl, :, :])
                yg = ypool.tile([C, GROUP, N], mybir.dt.bfloat16)
                for dd in range(GROUP):
                    ps = pspool.tile([C, N], mybir.dt.float32)
                    for k in range(J):
                        ncols = (J - k) * B
                        nc.tensor.matmul(
                            ps[:, k * B:],
                            wt[:, dd, k, :],
                            xg[:, dd, :ncols],
                            start=(k == 0),
                            stop=(k == J - 1),
                        )
                    nc.vector.tensor_copy(out=yg[:, dd, :], in_=ps[:])
                eng_b.dma_start(out=yt[:, sl, :], in_=yg)

    nc.compile()
    _CACHE["nc"] = nc
    return nc


def _prep_core_inputs(x, f, core):
    ds = slice(core * DC, (core + 1) * DC)
    xs = x[:, ds, :].reshape(B, DC, J, C).transpose(3, 1, 2, 0).reshape(C, DC, N)
    xt = np.ascontiguousarray(xs).astype(BF16)

    # fpad[d, 127 + t] = f[d, t]; ft[c, d, m] = fpad[d, 127 + m - c]
    fpad = np.zeros((DC, 127 + L), dtype=BF16)
    fpad[:, 127:] = f[ds].astype(BF16)
    base = fpad[:, 127:]
    sv = np.lib.stride_tricks.as_strided(
        base,
        shape=(C, DC, L),
        strides=(-fpad.strides[1], fpad.strides[0], fpad.strides[1]),
    )
    ft = np.ascontiguousarray(sv).reshape(C, DC, J, C)
    return {"xt": xt, "ft": ft}


def _run(x, f, trace=False):
    from concourse.bass_utils import run_bass_kernel_spmd

    nc = _build_nc()
    in_maps = [_prep_core_inputs(x, f, i) for i in range(NCORES)]
    res = run_bass_kernel_spmd(
        nc, in_maps, core_ids=list(range(NCORES)), trace=trace
    )

    y = np.empty((B, D, L), dtype=np.float32)
    for i in range(NCORES):
        ytc = np.asarray(res.results[i]["yt"]).astype(np.float32)  # [C(a), DC, N]
        ys = ytc.reshape(C, DC, J, B).transpose(3, 1, 2, 0).reshape(B, DC, L)
        y[:, i * DC:(i + 1) * DC, :] = ys
    return y, res


def kernel(x, filter):
    x = np.asarray(x, dtype=np.float32)
    f = np.asarray(filter, dtype=np.float32)
    y, _ = _run(x, f, trace=False)
    return y


# revision 10
# speedup vs baseline: 1.1122x; 1.1122x over previous
"""FFT causal long-conv (H3/Hyena fftconv) as a blocked-Toeplitz matmul kernel
for 8 Trainium2 NeuronCores.

Math: y[b,d,l] = sum_{t<=l} filter[d,t] * x[b,d,l-t]  (causal conv, L taps).

Instead of an on-device FFT, the causal conv is computed directly as a
lower-block-triangular Toeplitz matmul: with 128-wide blocks (J=L/128 blocks),
y_i = sum_{k<=i} T_k @ x_{i-k} where T_k[a,c] = f[128k + a - c].  The T_k are
materialized host-side (bf16) as PE-ready lhsT tiles, so the device does only
dense [128,128]x[128,N] matmuls accumulating in fp32 PSUM — no transposes,
no twiddles.  MAC count is L^2/2 per (b,d) pair, which at L=4096 beats a
two-stage FFT factorization, and every DMA is a clean contiguous transfer.

Sharding: channels D=1024 split 128 per core (data-parallel over D, per the
independence of each channel's conv); all B=16 batches stay on-core so each
matmul gets the full N=512 free dim.
"""

import numpy as np
import ml_dtypes

B, D, L = 16, 1024, 4096
NCORES = 8
DC = D // NCORES  # channels per core
C = 128           # time-block size == PE contraction dim
J = L // C        # 32 time blocks
N = J * B         # 512 = matmul free dim (j-block outer, batch inner)
GROUP = 4         # channels per DMA batch

BF16 = ml_dtypes.bfloat16

_CACHE = {}


def _build_nc():
    if "nc" in _CACHE:
        return _CACHE["nc"]

    import concourse.bacc as bacc
    import concourse.tile as tile
    import concourse.mybir as mybir

    nc = bacc.Bacc("TRN2", target_bir_lowering=False, debug=False, num_devices=NCORES)

    # Layouts are chosen so every DMA has long contiguous per-partition runs:
    #   xt[c, d, n]    n = j*B + b         (input, time-within-block on partitions)
    #   ft[c, d, k, a] = f[d, 128k + a - c] (PE-ready lhsT Toeplitz tiles)
    #   yt[a, d, n]    n = i*B + b         (output)
    xt = nc.dram_tensor("xt", [C, DC, N], mybir.dt.bfloat16, kind="ExternalInput")
    ft = nc.dram_tensor("ft", [C, DC, J, C], mybir.dt.bfloat16, kind="ExternalInput")
    yt = nc.dram_tensor("yt", [C, DC, N], mybir.dt.bfloat16, kind="ExternalOutput")

    with tile.TileContext(nc) as tc:
        with (
            tc.tile_pool(name="wpool", bufs=3) as wpool,
            tc.tile_pool(name="xpool", bufs=3) as xpool,
            tc.tile_pool(name="ypool", bufs=3) as ypool,
            tc.tile_pool(name="pspool", bufs=8, space="PSUM") as pspool,
        ):
            for g in range(DC // GROUP):
                sl = slice(g * GROUP, (g + 1) * GROUP)
                # Keep both HWDGE rings (SP + ACT) continuously busy: each
                # group's weight load is split half/half across the rings.
                # Everything stays off the slow gpsimd SWDGE path.
                eng_a = nc.sync if g % 2 == 0 else nc.scalar
                eng_b = nc.scalar if g % 2 == 0 else nc.sync
                xg = xpool.tile([C, GROUP, N], mybir.dt.bfloat16)
                eng_b.dma_start(out=xg, in_=xt[:, sl, :])
                wt = wpool.tile([C, GROUP, J, C], mybir.dt.bfloat16)
                h = GROUP // 2
                eng_a.dma_start(out=wt[:, :h], in_=ft[:, sl.start:sl.start + h, :, :])
                eng_b.dma_start(out=wt[:, h:], in_=ft[:, sl.start + h:sl.stop, :, :])
                yg = ypool.tile([C, GROUP, N], mybir.dt.bfloat16)
                for dd in range(GROUP):
                    ps = pspool.tile([C, N], mybir.dt.float32)
                    for k in range(J):
                        ncols = (J - k) * B
                        nc.tensor.matmul(
                            ps[:, k * B:],
                            wt[:, dd, k, :],
                            xg[:, dd, :ncols],
                            start=(k == 0),
                            stop=(k == J - 1),
                        )
                    nc.vector.tensor_copy(out=yg[:, dd, :], in_=ps[:])
                eng_b.dma_start(out=yt[:, sl, :], in_=yg)

    nc.compile()
    _CACHE["nc"] = nc
    return nc


def _prep_core_inputs(x, f, core):
    ds = slice(core * DC, (core + 1) * DC)
    xs = x[:, ds, :].reshape(B, DC, J, C).transpose(3, 1, 2, 0).reshape(C, DC, N)
    xt = np.ascontiguousarray(xs).astype(BF16)

    # fpad[d, 127 + t] = f[d, t]; ft[c, d, m] = fpad[d, 127 + m - c]
    fpad = np.zeros((DC, 127 + L), dtype=BF16)
    fpad[:, 127:] = f[ds].astype(BF16)
    base = fpad[:, 127:]
    sv = np.lib.stride_tricks.as_strided(
        base,
        shape=(C, DC, L),
        strides=(-fpad.strides[1], fpad.strides[0], fpad.strides[1]),
    )
    ft = np.ascontiguousarray(sv).reshape(C, DC, J, C)
    return {"xt": xt, "ft": ft}


def _run(x, f, trace=False):
    from concourse.bass_utils import run_bass_kernel_spmd

    nc = _build_nc()
    in_maps = [_prep_core_inputs(x, f, i) for i in range(NCORES)]
    res = run_bass_kernel_spmd(
        nc, in_maps, core_ids=list(range(NCORES)), trace=trace
    )

    y = np.empty((B, D, L), dtype=np.float32)
    for i in range(NCORES):
        ytc = np.asarray(res.results[i]["yt"]).astype(np.float32)  # [C(a), DC, N]
        ys = ytc.reshape(C, DC, J, B).transpose(3, 1, 2, 0).reshape(B, DC, L)
        y[:, i * DC:(i + 1) * DC, :] = ys
    return y, res


def kernel(x, filter):
    x = np.asarray(x, dtype=np.float32)
    f = np.asarray(filter, dtype=np.float32)
    y, _ = _run(x, f, trace=False)
    return y
